# revision 7
# baseline (speedup 1.0000x reference)
"""Multi-head causal attention (B=2, T=2048, E=1024, H=16, D=64) on 8 trn2 cores.

Sharding: tensor-parallel over heads — core c owns heads {2c, 2c+1} (a 128-wide
slice of the hidden dim). Each core computes q/k/v projections for its heads
over the full sequence, causal attention, and a partial output projection
(contraction over its 128 rows of Wo). The host sums the 8 bf16 partials + bias.

Per-core device program (SPMD — one NEFF, different weight slices per core):
  projections: QT/KT = (W.T @ xT) in [dim, token] layout; V in natural
    [token, dim|1] layout (the ones column makes P@V emit Z = sum(exp) free).
    Batch-0 projections run up front (fine-grained x DMAs for a fast start);
    batch-1 projection units are deferred into batch-0's attention waves as
    PE filler.
  attention, per (batch, 512-wide tq chunk), one 128-row tk block per wave:
    S^T for both heads into one [128, 2, 512] PSUM tile; exp on ScalarE in a
    single call covering both heads, trimmed to the causally valid column
    range; diagonal-block triangle mask multiplied on VectorE (bf16 2x mode);
    O^T|Z accumulated per head with causality-trimmed ranges.
  normalize: 1/Z (fp16) per head, K=1 ones-matmul broadcast of 1/Z across the
    64 head dims, then VectorE multiply reading both PSUM operands directly.
  output: out[tq, :] = O^T.T @ Wo_slice per 128 rows, staged to bf16 SBUF on
    the (otherwise idle) GpSimd engine, DMA'd as bf16. Output-projection units
    are deferred and popped into later attention waves so the ScalarE-bound
    batch-1 chunks keep the PE busy.

Timing signal is concourse's TimelineSim cost model (no NTFF profiling under
this axon client). Cost-model notes that shaped this schedule: PE drops to
half clock for 3us after any idle gap; matmul cost = out-free-size x
cycles/row (Ldweights free); engine op cost = free-size x cycle (partition
dim is free parallelism); DMA engines are one shared 360GB/s device.
"""

import numpy as np
import ml_dtypes

import concourse.bass as bass
import concourse.tile as tile
from concourse import bacc, mybir
from concourse.bass_utils import run_bass_kernel_spmd
from contextlib import ExitStack
from collections import deque

B, T, E, H, D = 2, 2048, 1024, 16, 64
BT = B * T            # 4096 tokens total
NCORE = 8
KC = E // 128         # contraction chunks for projections = 8
CQ = 512              # tq chunk width
NQB = T // CQ         # tq chunks per batch = 4
NKB = T // 128        # tk blocks per batch = 16

F32 = mybir.dt.float32
BF16 = mybir.dt.bfloat16
FP16 = mybir.dt.float16
AF = mybir.ActivationFunctionType

_cache = {}

# engine routing knobs (tuned against TimelineSim traces). GpSimd (Pool)
# cannot touch PSUM (BIR verifier rejects it), so PSUM->SBUF copies go to
# DVE/Act; Pool gets the SBUF-only triangle masks.
OST_ENGINE = "vector"    # output staging copies
B1_COPY_ENGINE = "vector"  # batch-1 projection psum->sbuf copies
MASK_ENGINE = "pool"     # diagonal-block triangle masks (SBUF-only)
ZB_DIRECT = False        # two PSUM reads in one op are illegal (NCC_IBVF027)


def _build():
    nc = bacc.Bacc("TRN2", target_bir_lowering=False, debug=False,
                   num_devices=NCORE)

    xT = nc.dram_tensor("xT", [E, BT], BF16, kind="ExternalInput").ap()
    wq = nc.dram_tensor("wq", [128, E], BF16, kind="ExternalInput").ap()
    wk = nc.dram_tensor("wk", [128, E], BF16, kind="ExternalInput").ap()
    wv = nc.dram_tensor("wv", [128, E], BF16, kind="ExternalInput").ap()
    wo = nc.dram_tensor("wo", [128, E], BF16, kind="ExternalInput").ap()
    tri = nc.dram_tensor("tri", [128, 2, 128], BF16, kind="ExternalInput").ap()
    out = nc.dram_tensor("out", [BT, E], BF16, kind="ExternalOutput").ap()

    with tile.TileContext(nc) as tc, ExitStack() as ctx:
        pers = ctx.enter_context(tc.tile_pool(name="pers", bufs=1))

        wq_sb = pers.tile([128, KC, 128], BF16, tag="wq")
        wk_sb = pers.tile([128, KC, 128], BF16, tag="wk")
        wv_sb = pers.tile([128, KC, 128], BF16, tag="wv")
        wo_sb = pers.tile([128, E], BF16, tag="wo")
        tri_sb = pers.tile([128, 2, 128], BF16, tag="tri")
        ones_sb = pers.tile([128, 64], FP16, tag="ones")
        qt_sb = pers.tile([128, BT], BF16, tag="qt")    # [dims(2 heads), tok]
        kt_sb = pers.tile([128, BT], BF16, tag="kt")
        # V natural + ones col per head: [tok%128, blk, h, d|1]
        v_sb = pers.tile([128, B * NKB, 2, 65], BF16, tag="v")
        ot_sb = pers.tile([128, BT], BF16, tag="ot")    # attn out, [dims, tok]

        xa_pool = ctx.enter_context(tc.tile_pool(name="xa", bufs=16))
        xb_pool = ctx.enter_context(tc.tile_pool(name="xb", bufs=8))
        sc_pool = ctx.enter_context(tc.tile_pool(name="sc", bufs=2, space="PSUM"))
        pv_pool = ctx.enter_context(tc.tile_pool(name="pv", bufs=2, space="PSUM"))
        vps = ctx.enter_context(tc.tile_pool(name="vps", bufs=1, space="PSUM"))
        zb_pool = ctx.enter_context(tc.tile_pool(name="zb", bufs=1, space="PSUM"))
        pt_pool = ctx.enter_context(tc.tile_pool(name="pt", bufs=4))
        zr_pool = ctx.enter_context(tc.tile_pool(name="zr", bufs=2))
        zbs_pool = ctx.enter_context(tc.tile_pool(name="zbs", bufs=2))
        ost_pool = ctx.enter_context(tc.tile_pool(name="ost", bufs=4))

        nc.vector.memset(ones_sb[:], 1.0)
        nc.vector.memset(v_sb[:, :, :, 64:65], 1.0)

        def copy_by(eng, dst, src):
            if eng == "act":
                nc.scalar.copy(dst, src)
            elif eng == "pool":
                nc.gpsimd.tensor_copy(dst, src)
            else:
                nc.vector.tensor_copy(dst, src)

        def qk_unit(w_sb, dst_sb, xts, xoff, t_, eng):
            def emit():
                sct = sc_pool.tile([128, 2, CQ], F32, tag="sc",
                                   name=f"qkps{t_}_{id(w_sb)}")
                ps = sct[:, 0, :]
                for kc in range(KC):
                    nc.tensor.matmul(ps, w_sb[:, kc],
                                     xts[kc][:, xoff:xoff + CQ],
                                     start=(kc == 0), stop=(kc == KC - 1))
                copy_by(eng, dst_sb[:, t_ * CQ:(t_ + 1) * CQ], ps)
            return emit

        def v_unit(xts, xoff, t_, eng):
            def emit():
                v_ps = vps.tile([128, CQ], F32, tag="v", name=f"vps{t_}")
                for j in range(CQ // 128):
                    jf = xoff + j * 128
                    for kc in range(KC):
                        nc.tensor.matmul(
                            v_ps[:, j * 128:(j + 1) * 128],
                            xts[kc][:, jf:jf + 128],
                            wv_sb[:, kc], start=(kc == 0),
                            stop=(kc == KC - 1))
                b4 = t_ * (CQ // 128)
                copy_by(eng, v_sb[:, b4:b4 + 4, :, 0:64],
                        v_ps[:].rearrange("p (j h v) -> p j h v", j=4, h=2))
            return emit

        # ---- weight + batch-0 x DMAs, ordered for a fast start ----
        xa = []
        for pair in range(2):
            xa.append([xa_pool.tile([128, 2 * CQ], BF16, tag="xa",
                                    name=f"xa{pair}_{kc}")
                       for kc in range(KC)])
        nc.sync.dma_start(wq_sb[:], wq.rearrange("p (kc d) -> p kc d", kc=KC))
        nc.sync.dma_start(xa[0][0][:], xT[0:128, 0:2 * CQ])
        nc.sync.dma_start(wk_sb[:], wk.rearrange("p (kc d) -> p kc d", kc=KC))
        nc.sync.dma_start(wv_sb[:], wv.rearrange("p (kc d) -> p kc d", kc=KC))
        for kc in range(1, KC):
            nc.sync.dma_start(xa[0][kc][:],
                              xT[kc * 128:(kc + 1) * 128, 0:2 * CQ])

        # batch-0 pair-0 projections inline (chunks t=0,1)
        for u in (qk_unit(wq_sb, qt_sb, xa[0], 0, 0, "act"),
                  qk_unit(wk_sb, kt_sb, xa[0], 0, 0, "act"),
                  v_unit(xa[0], 0, 0, "act")):
            u()
        for kc in range(KC):
            nc.sync.dma_start(xa[1][kc][:],
                              xT[kc * 128:(kc + 1) * 128, 2 * CQ:4 * CQ])
        nc.sync.dma_start(tri_sb[:], tri[:])
        nc.sync.dma_start(wo_sb[:], wo[:])
        for u in (qk_unit(wq_sb, qt_sb, xa[0], CQ, 1, "act"),
                  qk_unit(wk_sb, kt_sb, xa[0], CQ, 1, "act"),
                  v_unit(xa[0], CQ, 1, "act")):
            u()
        # batch-0 pair-1 projections inline (chunks t=2,3)
        for hf in range(2):
            for u in (qk_unit(wq_sb, qt_sb, xa[1], hf * CQ, 2 + hf, "act"),
                      qk_unit(wk_sb, kt_sb, xa[1], hf * CQ, 2 + hf, "act"),
                      v_unit(xa[1], hf * CQ, 2 + hf, "act")):
                u()

        # ---- batch-1 x DMAs (coarse) + deferred projection units ----
        xb = [xb_pool.tile([128, 4 * CQ], BF16, tag="xb", name=f"xb_{kc}")
              for kc in range(KC)]
        for kc in range(KC):
            nc.sync.dma_start(xb[kc][:],
                              xT[kc * 128:(kc + 1) * 128, 4 * CQ:8 * CQ])

        filler = deque()
        for t_ in range(4, 8):
            xoff = (t_ - 4) * CQ
            filler.append(qk_unit(wq_sb, qt_sb, xb, xoff, t_, B1_COPY_ENGINE))
            filler.append(qk_unit(wk_sb, kt_sb, xb, xoff, t_, B1_COPY_ENGINE))
            filler.append(v_unit(xb, xoff, t_, B1_COPY_ENGINE))

        def pop():
            if filler:
                filler.popleft()()

        def outproj_unit(b, cq, j):
            tqg = b * T + cq * CQ + j * 128
            def emit():
                o = sc_pool.tile([128, 2, CQ], F32, tag="sc",
                                 name=f"o_{b}_{cq}_{j}")
                for eh in range(2):
                    nc.tensor.matmul(o[:, eh, :], ot_sb[:, tqg:tqg + 128],
                                     wo_sb[:, eh * CQ:(eh + 1) * CQ],
                                     start=True, stop=True)
                ost = ost_pool.tile([128, 2, CQ], BF16, tag="ost",
                                    name=f"ost_{b}_{cq}_{j}")
                copy_by(OST_ENGINE, ost[:], o[:])
                nc.sync.dma_start(out[tqg:tqg + 128, :],
                                  ost[:].rearrange("p a b -> p (a b)"))
            return emit

        def attention_chunk(b, cq, pop_every):
            tb = b * T
            tq0 = cq * CQ
            nblk = (tq0 + CQ) // 128
            pv = [pv_pool.tile([128, CQ], F32, tag="pv",
                               name=f"pv{h}_{b}_{cq}") for h in range(2)]
            for kb in range(nblk):
                tk0 = kb * 128
                f0 = max(tk0 - tq0, 0)
                sct = sc_pool.tile([128, 2, CQ], F32, tag="sc",
                                   name=f"sc_{b}_{cq}_{kb}")
                for h in range(2):
                    hs = slice(h * 64, (h + 1) * 64)
                    nc.tensor.matmul(
                        sct[:, h, f0:], kt_sb[hs, tb + tk0:tb + tk0 + 128],
                        qt_sb[hs, tb + tq0 + f0:tb + tq0 + CQ],
                        start=True, stop=True)
                ptt = pt_pool.tile([128, 2, CQ], BF16, tag="pt",
                                   name=f"pt_{b}_{cq}_{kb}")
                nc.scalar.activation(ptt[:, :, f0:], sct[:, :, f0:],
                                     AF.Exp, scale=float(D) ** -0.5)
                s = tk0 - tq0
                if 0 <= s < CQ:  # diagonal: triangle mask, both heads at once
                    m_eng = nc.gpsimd if MASK_ENGINE == "pool" else nc.vector
                    m_eng.tensor_mul(ptt[:, :, s:s + 128],
                                     ptt[:, :, s:s + 128], tri_sb[:])
                for h in range(2):
                    nc.tensor.matmul(
                        pv[h][0:65, f0:], v_sb[:, b * NKB + kb, h],
                        ptt[:, h, f0:],
                        start=(kb == 0), stop=(kb == nblk - 1))
                if kb % pop_every == pop_every - 1:
                    pop()

            # normalize: 1/Z per head, ones-matmul broadcast, multiply
            zr = zr_pool.tile([128, 2, CQ], FP16, tag="zr", name=f"zr_{b}_{cq}")
            zb_ps = zb_pool.tile([128, CQ], F32, tag="zb", name=f"zb_{b}_{cq}")
            for h in range(2):
                with nc.allow_low_precision(
                        reason="1/Z in fp16 (2.4e-4 rel) feeds the K=1 "
                               "broadcast matmul at full PE rate"):
                    nc.vector.reciprocal(zr[64:65, h, :], pv[h][64:65, :])
                nc.tensor.matmul(zb_ps[h * 64:(h + 1) * 64, :],
                                 ones_sb[64:65, :], zr[64:65, h, :],
                                 start=True, stop=True)
            if ZB_DIRECT:
                zb_rd = zb_ps
            else:
                zb_rd = zbs_pool.tile([128, CQ], F32, tag="zbs",
                                      name=f"zbs_{b}_{cq}")
                nc.scalar.copy(zb_rd[:], zb_ps[:])
            for h in range(2):
                nc.vector.tensor_mul(
                    ot_sb[h * 64:(h + 1) * 64, tb + tq0:tb + tq0 + CQ],
                    pv[h][0:64, :], zb_rd[h * 64:(h + 1) * 64, :])
            for j in range(CQ // 128):
                filler.append(outproj_unit(b, cq, j))

        for cq in range(NQB):          # batch 0: filler = b1 proj + outproj
            attention_chunk(0, cq, 2)
        for cq in range(NQB):          # batch 1: filler = deferred outproj
            attention_chunk(1, cq, 1)
        while filler:
            filler.popleft()()

    nc.compile()
    return nc


def _host_prep(x, Wq, Wk, Wv, Wo):
    bf = ml_dtypes.bfloat16
    xT = np.ascontiguousarray(
        np.asarray(x, dtype=np.float32).reshape(BT, E).T).astype(bf)

    # tri[p, h, f] = 1 where kept (f >= p), applied to the diagonal 128x128
    # sub-block of P^T (tk on partitions, tq on free), both heads
    p = np.arange(128)[:, None]
    f = np.arange(128)[None, :]
    tri = np.broadcast_to((f >= p).astype(bf)[:, None, :], (128, 2, 128))
    tri = np.ascontiguousarray(tri)

    def perm(w):
        # [E, 128] -> [128p, kc, 128d] flattened: w[kc*128+p, d] -> out[p, kc, d]
        return np.ascontiguousarray(
            w.reshape(KC, 128, 128).transpose(1, 0, 2).reshape(128, E)).astype(bf)

    Wq = np.asarray(Wq, dtype=np.float32)
    Wk = np.asarray(Wk, dtype=np.float32)
    Wv = np.asarray(Wv, dtype=np.float32)
    Wo = np.asarray(Wo, dtype=np.float32)

    in_maps = []
    for c in range(NCORE):
        sl = slice(c * 128, (c + 1) * 128)
        in_maps.append({
            "xT": xT,
            "wq": perm(Wq[:, sl]),
            "wk": perm(Wk[:, sl]),
            "wv": perm(Wv[:, sl]),
            "wo": np.ascontiguousarray(Wo[sl, :]).astype(bf),
            "tri": tri,
        })
    return in_maps


def kernel(x, Wq, Wk, Wv, Wo, bo, _trace=False, _trace_kwargs=None):
    if "nc" not in _cache:
        _cache["nc"] = _build()
    nc = _cache["nc"]

    in_maps = _host_prep(x, Wq, Wk, Wv, Wo)
    kw = {}
    if _trace:
        kw = dict(trace=True, trace_cores=[0], **(_trace_kwargs or {}))
    res = run_bass_kernel_spmd(nc, in_maps, core_ids=list(range(NCORE)), **kw)
    _cache["last_result"] = res

    total = np.zeros((BT, E), dtype=np.float32)
    for r in res.results:
        total += np.asarray(r["out"], dtype=np.float32)
    total += np.asarray(bo, dtype=np.float32)[None, :]
    return total.reshape(B, T, E)


# revision 12
# speedup vs baseline: 1.1364x; 1.1364x over previous
"""Multi-head causal attention (B=2, T=2048, E=1024, H=16, D=64) on 8 trn2 cores.

Sharding: tensor-parallel over heads — core c owns heads {2c, 2c+1} (a 128-wide
slice of the hidden dim). Each core computes q/k/v projections for its heads
over the full sequence, causal attention, and a partial output projection
(contraction over its 128 rows of Wo). The host sums the 8 bf16 partials + bias.

Per-core device program (SPMD — one NEFF, different weight slices per core):
  projections: QT/KT = (W.T @ xT) in [dim, token] layout; V in natural
    [token, dim|1] layout (the ones column makes P@V emit Z = sum(exp) free).
    Batch-0 projections run up front (fine-grained x DMAs for a fast start);
    batch-1 projection units are deferred into batch-0's attention waves as
    PE filler.
  attention, per (batch, 512-wide tq chunk), one 128-row tk block per wave:
    S^T for both heads into one [128, 2, 512] PSUM tile; exp on ScalarE in a
    single call covering both heads, trimmed to the causally valid column
    range; diagonal-block triangle mask multiplied on VectorE (bf16 2x mode);
    O^T|Z accumulated per head with causality-trimmed ranges.
  normalize: 1/Z (fp16) per head, K=1 ones-matmul broadcast of 1/Z across the
    64 head dims, then VectorE multiply reading both PSUM operands directly.
  output: out[tq, :] = O^T.T @ Wo_slice per 128 rows, staged to bf16 SBUF on
    the (otherwise idle) GpSimd engine, DMA'd as bf16. Output-projection units
    are deferred and popped into later attention waves so the ScalarE-bound
    batch-1 chunks keep the PE busy.

Timing signal is concourse's TimelineSim cost model (no NTFF profiling under
this axon client). Cost-model notes that shaped this schedule: PE drops to
half clock for 3us after any idle gap; matmul cost = out-free-size x
cycles/row (Ldweights free); engine op cost = free-size x cycle (partition
dim is free parallelism); DMA engines are one shared 360GB/s device.
"""

import numpy as np
import ml_dtypes

import concourse.bass as bass
import concourse.tile as tile
from concourse import bacc, mybir
from concourse.bass_utils import run_bass_kernel_spmd
from contextlib import ExitStack
from collections import deque

B, T, E, H, D = 2, 2048, 1024, 16, 64
BT = B * T            # 4096 tokens total
NCORE = 8
KC = E // 128         # contraction chunks for projections = 8
CQ = 512              # tq chunk width
NQB = T // CQ         # tq chunks per batch = 4
NKB = T // 128        # tk blocks per batch = 16

F32 = mybir.dt.float32
BF16 = mybir.dt.bfloat16
FP16 = mybir.dt.float16
AF = mybir.ActivationFunctionType

_cache = {}

# engine routing knobs (tuned against TimelineSim traces). GpSimd (Pool)
# cannot touch PSUM (BIR verifier rejects it), so PSUM->SBUF copies go to
# DVE/Act; Pool gets the SBUF-only triangle masks.
OST_ENGINE = "vector"    # output staging copies
B1_COPY_ENGINE = "vector"  # batch-1 projection psum->sbuf copies
MASK_ENGINE = "pool"     # diagonal-block triangle masks (SBUF-only)
ZB_DIRECT = False        # two PSUM reads in one op are illegal (NCC_IBVF027)


def _build():
    nc = bacc.Bacc("TRN2", target_bir_lowering=False, debug=False,
                   num_devices=NCORE)

    xT = nc.dram_tensor("xT", [E, BT], BF16, kind="ExternalInput").ap()
    wq = nc.dram_tensor("wq", [128, E], BF16, kind="ExternalInput").ap()
    wk = nc.dram_tensor("wk", [128, E], BF16, kind="ExternalInput").ap()
    wv = nc.dram_tensor("wv", [128, E], BF16, kind="ExternalInput").ap()
    wo = nc.dram_tensor("wo", [128, E], BF16, kind="ExternalInput").ap()
    tri = nc.dram_tensor("tri", [128, 2, 128], BF16, kind="ExternalInput").ap()
    out = nc.dram_tensor("out", [BT, E], BF16, kind="ExternalOutput").ap()

    with tile.TileContext(nc) as tc, ExitStack() as ctx:
        pers = ctx.enter_context(tc.tile_pool(name="pers", bufs=1))

        wq_sb = pers.tile([128, KC, 128], BF16, tag="wq")
        wk_sb = pers.tile([128, KC, 128], BF16, tag="wk")
        wv_sb = pers.tile([128, KC, 128], BF16, tag="wv")
        wo_sb = pers.tile([128, E], BF16, tag="wo")
        tri_sb = pers.tile([128, 2, 128], BF16, tag="tri")
        ones_sb = pers.tile([128, 64], FP16, tag="ones")
        qt_sb = pers.tile([128, BT], BF16, tag="qt")    # [dims(2 heads), tok]
        kt_sb = pers.tile([128, BT], BF16, tag="kt")
        # V natural + ones col per head: [tok%128, blk, h, d|1]
        v_sb = pers.tile([128, B * NKB, 2, 65], BF16, tag="v")
        ot_sb = pers.tile([128, BT], BF16, tag="ot")    # attn out, [dims, tok]

        xa_pool = ctx.enter_context(tc.tile_pool(name="xa", bufs=16))
        xb_pool = ctx.enter_context(tc.tile_pool(name="xb", bufs=8))
        # PSUM budget (8 banks): sc ring 2x[128,2,512] = 4, pv 2x[128,512] = 2,
        # aux ring 2x[128,512] = 2 shared by V-units / zb broadcast / outproj.
        sc_pool = ctx.enter_context(tc.tile_pool(name="sc", bufs=2, space="PSUM"))
        pv_pool = ctx.enter_context(tc.tile_pool(name="pv", bufs=2, space="PSUM"))
        aux_pool = ctx.enter_context(tc.tile_pool(name="aux", bufs=2, space="PSUM"))
        pt_pool = ctx.enter_context(tc.tile_pool(name="pt", bufs=4))
        zr_pool = ctx.enter_context(tc.tile_pool(name="zr", bufs=2))
        zbs_pool = ctx.enter_context(tc.tile_pool(name="zbs", bufs=2))
        ost_pool = ctx.enter_context(tc.tile_pool(name="ost", bufs=4))

        nc.vector.memset(ones_sb[:], 1.0)
        nc.vector.memset(v_sb[:, :, :, 64:65], 1.0)

        def copy_by(eng, dst, src):
            if eng == "act":
                nc.scalar.copy(dst, src)
            elif eng == "pool":
                nc.gpsimd.tensor_copy(dst, src)
            else:
                nc.vector.tensor_copy(dst, src)

        def qk_unit(w_sb, dst_sb, xts, xoff, t_, eng):
            def emit():
                sct = sc_pool.tile([128, 2, CQ], F32, tag="sc",
                                   name=f"qkps{t_}_{id(w_sb)}")
                ps = sct[:, 0, :]
                for kc in range(KC):
                    nc.tensor.matmul(ps, w_sb[:, kc],
                                     xts[kc][:, xoff:xoff + CQ],
                                     start=(kc == 0), stop=(kc == KC - 1))
                copy_by(eng, dst_sb[:, t_ * CQ:(t_ + 1) * CQ], ps)
            return emit

        def v_unit(xts, xoff, t_, eng):
            def emit():
                v_ps = aux_pool.tile([128, CQ], F32, tag="aux", name=f"vps{t_}")
                for j in range(CQ // 128):
                    jf = xoff + j * 128
                    for kc in range(KC):
                        nc.tensor.matmul(
                            v_ps[:, j * 128:(j + 1) * 128],
                            xts[kc][:, jf:jf + 128],
                            wv_sb[:, kc], start=(kc == 0),
                            stop=(kc == KC - 1))
                b4 = t_ * (CQ // 128)
                copy_by(eng, v_sb[:, b4:b4 + 4, :, 0:64],
                        v_ps[:].rearrange("p (j h v) -> p j h v", j=4, h=2))
            return emit

        # ---- weight + batch-0 x DMAs, ordered for a fast start ----
        xa = []
        for pair in range(2):
            xa.append([xa_pool.tile([128, 2 * CQ], BF16, tag="xa",
                                    name=f"xa{pair}_{kc}")
                       for kc in range(KC)])
        nc.sync.dma_start(wq_sb[:], wq.rearrange("p (kc d) -> p kc d", kc=KC))
        nc.sync.dma_start(xa[0][0][:], xT[0:128, 0:2 * CQ])
        nc.sync.dma_start(wk_sb[:], wk.rearrange("p (kc d) -> p kc d", kc=KC))
        nc.sync.dma_start(wv_sb[:], wv.rearrange("p (kc d) -> p kc d", kc=KC))
        for kc in range(1, KC):
            nc.sync.dma_start(xa[0][kc][:],
                              xT[kc * 128:(kc + 1) * 128, 0:2 * CQ])

        # batch-0 pair-0 projections inline (chunks t=0,1)
        for u in (qk_unit(wq_sb, qt_sb, xa[0], 0, 0, "act"),
                  qk_unit(wk_sb, kt_sb, xa[0], 0, 0, "act"),
                  v_unit(xa[0], 0, 0, "act")):
            u()
        for kc in range(KC):
            nc.sync.dma_start(xa[1][kc][:],
                              xT[kc * 128:(kc + 1) * 128, 2 * CQ:4 * CQ])
        nc.sync.dma_start(tri_sb[:], tri[:])
        nc.sync.dma_start(wo_sb[:], wo[:])
        for u in (qk_unit(wq_sb, qt_sb, xa[0], CQ, 1, "act"),
                  qk_unit(wk_sb, kt_sb, xa[0], CQ, 1, "act"),
                  v_unit(xa[0], CQ, 1, "act")):
            u()
        # batch-0 pair-1 projections inline (chunks t=2,3)
        for hf in range(2):
            for u in (qk_unit(wq_sb, qt_sb, xa[1], hf * CQ, 2 + hf, "act"),
                      qk_unit(wk_sb, kt_sb, xa[1], hf * CQ, 2 + hf, "act"),
                      v_unit(xa[1], hf * CQ, 2 + hf, "act")):
                u()

        # ---- batch-1 x DMAs (coarse) + deferred projection units ----
        xb = [xb_pool.tile([128, 4 * CQ], BF16, tag="xb", name=f"xb_{kc}")
              for kc in range(KC)]
        for kc in range(KC):
            nc.sync.dma_start(xb[kc][:],
                              xT[kc * 128:(kc + 1) * 128, 4 * CQ:8 * CQ])

        filler = deque()
        for t_ in range(4, 8):
            xoff = (t_ - 4) * CQ
            filler.append(qk_unit(wq_sb, qt_sb, xb, xoff, t_, B1_COPY_ENGINE))
            filler.append(qk_unit(wk_sb, kt_sb, xb, xoff, t_, B1_COPY_ENGINE))
            filler.append(v_unit(xb, xoff, t_, B1_COPY_ENGINE))

        def pop():
            if filler:
                filler.popleft()()

        def outproj_unit(b, cq, j):
            tqg = b * T + cq * CQ + j * 128
            ost = [None]
            def half(eh):
                def emit():
                    o = aux_pool.tile([128, CQ], F32, tag="aux",
                                      name=f"o_{b}_{cq}_{j}_{eh}")
                    nc.tensor.matmul(o[:], ot_sb[:, tqg:tqg + 128],
                                     wo_sb[:, eh * CQ:(eh + 1) * CQ],
                                     start=True, stop=True)
                    if ost[0] is None:
                        ost[0] = ost_pool.tile([128, 2, CQ], BF16, tag="ost",
                                               name=f"ost_{b}_{cq}_{j}")
                    copy_by(OST_ENGINE, ost[0][:, eh, :], o[:])
                    if eh == 1:
                        nc.sync.dma_start(
                            out[tqg:tqg + 128, :],
                            ost[0][:].rearrange("p a b -> p (a b)"))
                return emit
            return [half(0), half(1)]

        def attention_chunk(b, cq, pop_every):
            tb = b * T
            tq0 = cq * CQ
            nblk = (tq0 + CQ) // 128
            pv = [pv_pool.tile([128, CQ], F32, tag="pv",
                               name=f"pv{h}_{b}_{cq}") for h in range(2)]
            for kb in range(nblk):
                tk0 = kb * 128
                f0 = max(tk0 - tq0, 0)
                sct = sc_pool.tile([128, 2, CQ], F32, tag="sc",
                                   name=f"sc_{b}_{cq}_{kb}")
                for h in range(2):
                    hs = slice(h * 64, (h + 1) * 64)
                    nc.tensor.matmul(
                        sct[:, h, f0:], kt_sb[hs, tb + tk0:tb + tk0 + 128],
                        qt_sb[hs, tb + tq0 + f0:tb + tq0 + CQ],
                        start=True, stop=True)
                ptt = pt_pool.tile([128, 2, CQ], BF16, tag="pt",
                                   name=f"pt_{b}_{cq}_{kb}")
                nc.scalar.activation(ptt[:, :, f0:], sct[:, :, f0:],
                                     AF.Exp, scale=float(D) ** -0.5)
                s = tk0 - tq0
                if 0 <= s < CQ:  # diagonal: triangle mask, both heads at once
                    m_eng = nc.gpsimd if MASK_ENGINE == "pool" else nc.vector
                    m_eng.tensor_mul(ptt[:, :, s:s + 128],
                                     ptt[:, :, s:s + 128], tri_sb[:])
                for h in range(2):
                    nc.tensor.matmul(
                        pv[h][0:65, f0:], v_sb[:, b * NKB + kb, h],
                        ptt[:, h, f0:],
                        start=(kb == 0), stop=(kb == nblk - 1))
                if kb % pop_every == pop_every - 1:
                    pop()

            # normalize: 1/Z per head, ones-matmul broadcast, multiply
            zr = zr_pool.tile([128, 2, CQ], FP16, tag="zr", name=f"zr_{b}_{cq}")
            zb_ps = aux_pool.tile([128, CQ], F32, tag="aux", name=f"zb_{b}_{cq}")
            for h in range(2):
                with nc.allow_low_precision(
                        reason="1/Z in fp16 (2.4e-4 rel) feeds the K=1 "
                               "broadcast matmul at full PE rate"):
                    nc.vector.reciprocal(zr[64:65, h, :], pv[h][64:65, :])
                nc.tensor.matmul(zb_ps[h * 64:(h + 1) * 64, :],
                                 ones_sb[64:65, :], zr[64:65, h, :],
                                 start=True, stop=True)
            if ZB_DIRECT:
                zb_rd = zb_ps
            else:
                zb_rd = zbs_pool.tile([128, CQ], F32, tag="zbs",
                                      name=f"zbs_{b}_{cq}")
                nc.scalar.copy(zb_rd[:], zb_ps[:])
            for h in range(2):
                nc.vector.tensor_mul(
                    ot_sb[h * 64:(h + 1) * 64, tb + tq0:tb + tq0 + CQ],
                    pv[h][0:64, :], zb_rd[h * 64:(h + 1) * 64, :])
            for j in range(CQ // 128):
                filler.extend(outproj_unit(b, cq, j))

        for cq in range(NQB):          # batch 0: filler = b1 proj + outproj
            attention_chunk(0, cq, 2)
        for cq in range(NQB):          # batch 1: filler = deferred outproj
            attention_chunk(1, cq, 1)
        while filler:
            filler.popleft()()

    nc.compile()
    return nc


def _host_prep(x, Wq, Wk, Wv, Wo):
    bf = ml_dtypes.bfloat16
    xT = np.ascontiguousarray(
        np.asarray(x, dtype=np.float32).reshape(BT, E).T).astype(bf)

    # tri[p, h, f] = 1 where kept (f >= p), applied to the diagonal 128x128
    # sub-block of P^T (tk on partitions, tq on free), both heads
    p = np.arange(128)[:, None]
    f = np.arange(128)[None, :]
    tri = np.broadcast_to((f >= p).astype(bf)[:, None, :], (128, 2, 128))
    tri = np.ascontiguousarray(tri)

    def perm(w):
        # [E, 128] -> [128p, kc, 128d] flattened: w[kc*128+p, d] -> out[p, kc, d]
        return np.ascontiguousarray(
            w.reshape(KC, 128, 128).transpose(1, 0, 2).reshape(128, E)).astype(bf)

    Wq = np.asarray(Wq, dtype=np.float32)
    Wk = np.asarray(Wk, dtype=np.float32)
    Wv = np.asarray(Wv, dtype=np.float32)
    Wo = np.asarray(Wo, dtype=np.float32)

    in_maps = []
    for c in range(NCORE):
        sl = slice(c * 128, (c + 1) * 128)
        in_maps.append({
            "xT": xT,
            "wq": perm(Wq[:, sl]),
            "wk": perm(Wk[:, sl]),
            "wv": perm(Wv[:, sl]),
            "wo": np.ascontiguousarray(Wo[sl, :]).astype(bf),
            "tri": tri,
        })
    return in_maps


def kernel(x, Wq, Wk, Wv, Wo, bo, _trace=False, _trace_kwargs=None):
    if "nc" not in _cache:
        _cache["nc"] = _build()
    nc = _cache["nc"]

    in_maps = _host_prep(x, Wq, Wk, Wv, Wo)
    kw = {}
    if _trace:
        kw = dict(trace=True, trace_cores=[0], **(_trace_kwargs or {}))
    res = run_bass_kernel_spmd(nc, in_maps, core_ids=list(range(NCORE)), **kw)
    _cache["last_result"] = res

    total = np.zeros((BT, E), dtype=np.float32)
    for r in res.results:
        total += np.asarray(r["out"], dtype=np.float32)
    total += np.asarray(bo, dtype=np.float32)[None, :]
    return total.reshape(B, T, E)


# revision 20
# speedup vs baseline: 1.1370x; 1.0005x over previous
"""Multi-head causal attention (B=2, T=2048, E=1024, H=16, D=64) on 8 trn2 cores.

Sharding: tensor-parallel over heads — core c owns heads {2c, 2c+1} (a 128-wide
slice of the hidden dim). Each core computes q/k/v projections for its heads
over the full sequence, causal attention, and a partial output projection
(contraction over its 128 rows of Wo). The host sums the 8 bf16 partials + bias.

Per-core device program (SPMD — one NEFF, different weight slices per core):
  projections: QT/KT = (W.T @ xT) in [dim, token] layout; V in natural
    [token, dim|1] layout (the ones column makes P@V emit Z = sum(exp) free).
    Batch-0 projections run up front (fine-grained x DMAs for a fast start);
    batch-1 projection units are deferred into batch-0's attention waves as
    PE filler.
  attention, per (batch, 512-wide tq chunk), one 128-row tk block per wave:
    S^T for both heads into one [128, 2, 512] PSUM tile; exp on ScalarE in a
    single call covering both heads, trimmed to the causally valid column
    range; diagonal-block triangle mask multiplied on VectorE (bf16 2x mode);
    O^T|Z accumulated per head with causality-trimmed ranges.
  normalize: 1/Z (fp16) per head, K=1 ones-matmul broadcast of 1/Z across the
    64 head dims, then VectorE multiply reading both PSUM operands directly.
  output: out[tq, :] = O^T.T @ Wo_slice per 128 rows, staged to bf16 SBUF on
    the (otherwise idle) GpSimd engine, DMA'd as bf16. Output-projection units
    are deferred and popped into later attention waves so the ScalarE-bound
    batch-1 chunks keep the PE busy.

Timing signal is concourse's TimelineSim cost model (no NTFF profiling under
this axon client). Cost-model notes that shaped this schedule: PE drops to
half clock for 3us after any idle gap; matmul cost = out-free-size x
cycles/row (Ldweights free); engine op cost = free-size x cycle (partition
dim is free parallelism); DMA engines are one shared 360GB/s device.
"""

import numpy as np
import ml_dtypes

import concourse.bass as bass
import concourse.tile as tile
from concourse import bacc, mybir
from concourse.bass_utils import run_bass_kernel_spmd
from contextlib import ExitStack
from collections import deque

B, T, E, H, D = 2, 2048, 1024, 16, 64
BT = B * T            # 4096 tokens total
NCORE = 8
KC = E // 128         # contraction chunks for projections = 8
CQ = 512              # tq chunk width
NQB = T // CQ         # tq chunks per batch = 4
NKB = T // 128        # tk blocks per batch = 16

F32 = mybir.dt.float32
BF16 = mybir.dt.bfloat16
FP16 = mybir.dt.float16
AF = mybir.ActivationFunctionType

_cache = {}

# engine routing knobs (tuned against TimelineSim traces). GpSimd (Pool)
# cannot touch PSUM (BIR verifier rejects it), so PSUM->SBUF copies go to
# DVE/Act; Pool gets the SBUF-only triangle masks.
OST_ENGINE = "vector"    # output staging copies
B1_COPY_ENGINE = "vector"  # batch-1 projection psum->sbuf copies
MASK_ENGINE = "vector"   # diagonal-block triangle masks (bf16 SBUF, 2x mode)
ZB_DIRECT = False        # two PSUM reads in one op are illegal (NCC_IBVF027)


def _build():
    nc = bacc.Bacc("TRN2", target_bir_lowering=False, debug=False,
                   num_devices=NCORE)

    xT = nc.dram_tensor("xT", [E, BT], BF16, kind="ExternalInput").ap()
    wq = nc.dram_tensor("wq", [128, E], BF16, kind="ExternalInput").ap()
    wk = nc.dram_tensor("wk", [128, E], BF16, kind="ExternalInput").ap()
    wv = nc.dram_tensor("wv", [128, E], BF16, kind="ExternalInput").ap()
    wo = nc.dram_tensor("wo", [128, E], BF16, kind="ExternalInput").ap()
    tri = nc.dram_tensor("tri", [128, 2, 128], BF16, kind="ExternalInput").ap()
    out = nc.dram_tensor("out", [BT, E], BF16, kind="ExternalOutput").ap()

    with tile.TileContext(nc) as tc, ExitStack() as ctx:
        pers = ctx.enter_context(tc.tile_pool(name="pers", bufs=1))

        wq_sb = pers.tile([128, KC, 128], BF16, tag="wq")
        wk_sb = pers.tile([128, KC, 128], BF16, tag="wk")
        wv_sb = pers.tile([128, KC, 128], BF16, tag="wv")
        wo_sb = pers.tile([128, E], BF16, tag="wo")
        tri_sb = pers.tile([128, 2, 128], BF16, tag="tri")
        ones_sb = pers.tile([128, 64], FP16, tag="ones")
        qt_sb = pers.tile([128, BT], BF16, tag="qt")    # [dims(2 heads), tok]
        kt_sb = pers.tile([128, BT], BF16, tag="kt")
        # V natural + ones col per head: [tok%128, blk, h, d|1]
        v_sb = pers.tile([128, B * NKB, 2, 65], BF16, tag="v")
        ot_sb = pers.tile([128, BT], BF16, tag="ot")    # attn out, [dims, tok]

        xa_pool = ctx.enter_context(tc.tile_pool(name="xa", bufs=16))
        xb_pool = ctx.enter_context(tc.tile_pool(name="xb", bufs=8))
        # PSUM budget (8 banks): sc ring 2x[128,2,512] = 4, pv 2x[128,512] = 2,
        # aux ring 2x[128,512] = 2 shared by V-units / zb broadcast / outproj.
        sc_pool = ctx.enter_context(tc.tile_pool(name="sc", bufs=2, space="PSUM"))
        pv_pool = ctx.enter_context(tc.tile_pool(name="pv", bufs=2, space="PSUM"))
        aux_pool = ctx.enter_context(tc.tile_pool(name="aux", bufs=2, space="PSUM"))
        pt_pool = ctx.enter_context(tc.tile_pool(name="pt", bufs=4))
        zr_pool = ctx.enter_context(tc.tile_pool(name="zr", bufs=2))
        zbs_pool = ctx.enter_context(tc.tile_pool(name="zbs", bufs=2))
        ost_pool = ctx.enter_context(tc.tile_pool(name="ost", bufs=4))

        nc.vector.memset(ones_sb[:], 1.0)
        nc.vector.memset(v_sb[:, :, :, 64:65], 1.0)

        def copy_by(eng, dst, src):
            if eng == "act":
                nc.scalar.copy(dst, src)
            elif eng == "pool":
                nc.gpsimd.tensor_copy(dst, src)
            else:
                nc.vector.tensor_copy(dst, src)

        def qk_unit(w_sb, dst_sb, xts, xoff, t_, eng):
            def emit(ctx=None):
                sct = sc_pool.tile([128, 2, CQ], F32, tag="sc",
                                   name=f"qkps{t_}_{id(w_sb)}")
                ps = sct[:, 0, :]
                for kc in range(KC):
                    nc.tensor.matmul(ps, w_sb[:, kc],
                                     xts[kc][:, xoff:xoff + CQ],
                                     start=(kc == 0), stop=(kc == KC - 1))
                copy_by(eng, dst_sb[:, t_ * CQ:(t_ + 1) * CQ], ps)
            return emit

        def v_unit(xts, xoff, t_, eng):
            def emit(ctx=None):
                v_ps = aux_pool.tile([128, CQ], F32, tag="aux", name=f"vps{t_}")
                for j in range(CQ // 128):
                    jf = xoff + j * 128
                    for kc in range(KC):
                        nc.tensor.matmul(
                            v_ps[:, j * 128:(j + 1) * 128],
                            xts[kc][:, jf:jf + 128],
                            wv_sb[:, kc], start=(kc == 0),
                            stop=(kc == KC - 1))
                b4 = t_ * (CQ // 128)
                copy_by(eng, v_sb[:, b4:b4 + 4, :, 0:64],
                        v_ps[:].rearrange("p (j h v) -> p j h v", j=4, h=2))
            return emit

        # ---- weight + batch-0 x DMAs, ordered for a fast start ----
        xa = []
        for pair in range(2):
            xa.append([xa_pool.tile([128, 2 * CQ], BF16, tag="xa",
                                    name=f"xa{pair}_{kc}")
                       for kc in range(KC)])
        # split the first weight chunk + first x chunk so the very first
        # matmul's inputs arrive ahead of the bulk transfers
        wq_r = wq.rearrange("p (kc d) -> p kc d", kc=KC)
        nc.sync.dma_start(wq_sb[:, 0], wq_r[:, 0])
        nc.sync.dma_start(xa[0][0][:, 0:CQ], xT[0:128, 0:CQ])
        nc.sync.dma_start(wq_sb[:, 1:], wq_r[:, 1:])
        nc.sync.dma_start(xa[0][0][:, CQ:], xT[0:128, CQ:2 * CQ])
        nc.sync.dma_start(wk_sb[:], wk.rearrange("p (kc d) -> p kc d", kc=KC))
        nc.sync.dma_start(wv_sb[:], wv.rearrange("p (kc d) -> p kc d", kc=KC))
        for kc in range(1, KC):
            nc.sync.dma_start(xa[0][kc][:],
                              xT[kc * 128:(kc + 1) * 128, 0:2 * CQ])

        # batch-0 pair-0 projections inline (chunks t=0,1)
        for u in (qk_unit(wq_sb, qt_sb, xa[0], 0, 0, "act"),
                  qk_unit(wk_sb, kt_sb, xa[0], 0, 0, "act"),
                  v_unit(xa[0], 0, 0, "act")):
            u()
        for kc in range(KC):
            nc.sync.dma_start(xa[1][kc][:],
                              xT[kc * 128:(kc + 1) * 128, 2 * CQ:4 * CQ])
        nc.sync.dma_start(tri_sb[:], tri[:])
        nc.sync.dma_start(wo_sb[:], wo[:])
        for u in (qk_unit(wq_sb, qt_sb, xa[0], CQ, 1, "act"),
                  qk_unit(wk_sb, kt_sb, xa[0], CQ, 1, "act"),
                  v_unit(xa[0], CQ, 1, "act")):
            u()
        # batch-0 pair-1 projections inline (chunks t=2,3)
        for hf in range(2):
            for u in (qk_unit(wq_sb, qt_sb, xa[1], hf * CQ, 2 + hf, "act"),
                      qk_unit(wk_sb, kt_sb, xa[1], hf * CQ, 2 + hf, "act"),
                      v_unit(xa[1], hf * CQ, 2 + hf, "act")):
                u()

        # ---- batch-1 x DMAs (coarse) + deferred projection units ----
        xb = [xb_pool.tile([128, 4 * CQ], BF16, tag="xb", name=f"xb_{kc}")
              for kc in range(KC)]
        for kc in range(KC):
            nc.sync.dma_start(xb[kc][:],
                              xT[kc * 128:(kc + 1) * 128, 4 * CQ:8 * CQ])

        filler = deque()
        for t_ in range(4, 8):
            xoff = (t_ - 4) * CQ
            filler.append(qk_unit(wq_sb, qt_sb, xb, xoff, t_, B1_COPY_ENGINE))
            filler.append(qk_unit(wk_sb, kt_sb, xb, xoff, t_, B1_COPY_ENGINE))
            filler.append(v_unit(xb, xoff, t_, B1_COPY_ENGINE))

        default_ctx = {"ring": aux_pool, "copy_eng": OST_ENGINE}

        def pop(n=1, ctx=default_ctx):
            for _ in range(n):
                if filler:
                    filler.popleft()(ctx)

        def outproj_unit(b, cq, j):
            tqg = b * T + cq * CQ + j * 128
            ost = [None]
            def half(eh):
                def emit(ctx):
                    ring = ctx["ring"]
                    if ring is sc_pool:
                        sct = sc_pool.tile([128, 2, CQ], F32, tag="sc",
                                           name=f"o_{b}_{cq}_{j}_{eh}")
                        o = sct[:, 0, :]
                    else:
                        o = ring.tile([128, CQ], F32, tag="aux",
                                      name=f"o_{b}_{cq}_{j}_{eh}")[:]
                    nc.tensor.matmul(o, ot_sb[:, tqg:tqg + 128],
                                     wo_sb[:, eh * CQ:(eh + 1) * CQ],
                                     start=True, stop=True)
                    if ost[0] is None:
                        ost[0] = ost_pool.tile([128, 2, CQ], BF16, tag="ost",
                                               name=f"ost_{b}_{cq}_{j}")
                    copy_by(ctx["copy_eng"], ost[0][:, eh, :], o)
                    if eh == 1:
                        nc.sync.dma_start(
                            out[tqg:tqg + 128, :],
                            ost[0][:].rearrange("p a b -> p (a b)"))
                return emit
            return [half(0), half(1)]

        def attention_chunk(b, cq, pop_every):
            tb = b * T
            tq0 = cq * CQ
            nblk = (tq0 + CQ) // 128
            pv = [pv_pool.tile([128, CQ], F32, tag="pv",
                               name=f"pv{h}_{b}_{cq}") for h in range(2)]
            for kb in range(nblk):
                tk0 = kb * 128
                f0 = max(tk0 - tq0, 0)
                sct = sc_pool.tile([128, 2, CQ], F32, tag="sc",
                                   name=f"sc_{b}_{cq}_{kb}")
                for h in range(2):
                    hs = slice(h * 64, (h + 1) * 64)
                    nc.tensor.matmul(
                        sct[:, h, f0:], kt_sb[hs, tb + tk0:tb + tk0 + 128],
                        qt_sb[hs, tb + tq0 + f0:tb + tq0 + CQ],
                        start=True, stop=True)
                ptt = pt_pool.tile([128, 2, CQ], BF16, tag="pt",
                                   name=f"pt_{b}_{cq}_{kb}")
                nc.scalar.activation(ptt[:, :, f0:], sct[:, :, f0:],
                                     AF.Exp, scale=float(D) ** -0.5)
                s = tk0 - tq0
                if 0 <= s < CQ:  # diagonal: triangle mask, both heads at once
                    m_eng = nc.gpsimd if MASK_ENGINE == "pool" else nc.vector
                    m_eng.tensor_mul(ptt[:, :, s:s + 128],
                                     ptt[:, :, s:s + 128], tri_sb[:])
                for h in range(2):
                    nc.tensor.matmul(
                        pv[h][0:65, f0:], v_sb[:, b * NKB + kb, h],
                        ptt[:, h, f0:],
                        start=(kb == 0), stop=(kb == nblk - 1))
                if 0 <= s < CQ:
                    pop(2)   # diagonal waves have little PE work of their own
                elif kb % pop_every == pop_every - 1:
                    pop()

            # normalize: 1/Z per head, ones-matmul broadcast, multiply
            zr = zr_pool.tile([128, 2, CQ], FP16, tag="zr", name=f"zr_{b}_{cq}")
            zb_ps = aux_pool.tile([128, CQ], F32, tag="aux", name=f"zb_{b}_{cq}")
            for h in range(2):
                with nc.allow_low_precision(
                        reason="1/Z in fp16 (2.4e-4 rel) feeds the K=1 "
                               "broadcast matmul at full PE rate"):
                    nc.vector.reciprocal(zr[64:65, h, :], pv[h][64:65, :])
                nc.tensor.matmul(zb_ps[h * 64:(h + 1) * 64, :],
                                 ones_sb[64:65, :], zr[64:65, h, :],
                                 start=True, stop=True)
            if ZB_DIRECT:
                zb_rd = zb_ps
            else:
                zb_rd = zbs_pool.tile([128, CQ], F32, tag="zbs",
                                      name=f"zbs_{b}_{cq}")
                nc.scalar.copy(zb_rd[:], zb_ps[:])
            for h in range(2):
                nc.vector.tensor_mul(
                    ot_sb[h * 64:(h + 1) * 64, tb + tq0:tb + tq0 + CQ],
                    pv[h][0:64, :], zb_rd[h * 64:(h + 1) * 64, :])
            for j in range(CQ // 128):
                filler.extend(outproj_unit(b, cq, j))

        for cq in range(NQB):          # batch 0: filler = b1 proj + outproj
            attention_chunk(0, cq, 2)
        for cq in range(NQB):          # batch 1: filler = deferred outproj
            attention_chunk(1, cq, 1)
        # flush: no attention left, so alternate PSUM rings and copy engines
        # to keep the remaining outproj units pipelined
        k = 0
        while filler:
            filler.popleft()({"ring": sc_pool if k % 2 else aux_pool,
                              "copy_eng": "act" if k % 2 else "vector"})
            k += 1

    nc.compile()
    return nc


def _host_prep(x, Wq, Wk, Wv, Wo):
    bf = ml_dtypes.bfloat16
    xT = np.ascontiguousarray(
        np.asarray(x, dtype=np.float32).reshape(BT, E).T).astype(bf)

    # tri[p, h, f] = 1 where kept (f >= p), applied to the diagonal 128x128
    # sub-block of P^T (tk on partitions, tq on free), both heads
    p = np.arange(128)[:, None]
    f = np.arange(128)[None, :]
    tri = np.broadcast_to((f >= p).astype(bf)[:, None, :], (128, 2, 128))
    tri = np.ascontiguousarray(tri)

    def perm(w):
        # [E, 128] -> [128p, kc, 128d] flattened: w[kc*128+p, d] -> out[p, kc, d]
        return np.ascontiguousarray(
            w.reshape(KC, 128, 128).transpose(1, 0, 2).reshape(128, E)).astype(bf)

    Wq = np.asarray(Wq, dtype=np.float32)
    Wk = np.asarray(Wk, dtype=np.float32)
    Wv = np.asarray(Wv, dtype=np.float32)
    Wo = np.asarray(Wo, dtype=np.float32)

    in_maps = []
    for c in range(NCORE):
        sl = slice(c * 128, (c + 1) * 128)
        in_maps.append({
            "xT": xT,
            "wq": perm(Wq[:, sl]),
            "wk": perm(Wk[:, sl]),
            "wv": perm(Wv[:, sl]),
            "wo": np.ascontiguousarray(Wo[sl, :]).astype(bf),
            "tri": tri,
        })
    return in_maps


def kernel(x, Wq, Wk, Wv, Wo, bo, _trace=False, _trace_kwargs=None):
    if "nc" not in _cache:
        _cache["nc"] = _build()
    nc = _cache["nc"]

    in_maps = _host_prep(x, Wq, Wk, Wv, Wo)
    kw = {}
    if _trace:
        kw = dict(trace=True, trace_cores=[0], **(_trace_kwargs or {}))
    res = run_bass_kernel_spmd(nc, in_maps, core_ids=list(range(NCORE)), **kw)
    _cache["last_result"] = res

    total = np.zeros((BT, E), dtype=np.float32)
    for r in res.results:
        total += np.asarray(r["out"], dtype=np.float32)
    total += np.asarray(bo, dtype=np.float32)[None, :]
    return total.reshape(B, T, E)
